# revision 26
# baseline (speedup 1.0000x reference)
"""Trainium2 Bass kernel for DSConvSpectral.

Contract: kernel(**inputs) takes FULL unsharded inputs (as produced by
setup_inputs()) and returns the FULL [2,64,360,720] float32 output.

Strategy (8 NeuronCores, SPMD):
- Shard latitude H into 8 contiguous blocks of 45 rows; each core also
  computes the 5-row halo on each side (duplicated work, no collectives).
- Per spectral row: dense rfft along W as bf16 matmuls against precomputed
  DFT matrices (x is host-pre-transposed so the contraction dim lands on
  partitions); pointwise chain (SpectralRescale folded into W1 on host,
  mod_relu, W2, complex GLU gate) split across DVE/ACT/GPSIMD; result
  spectrum rows stored in an SBUF-resident bf16 slab.
- Depthwise (11,1) H-conv done in the frequency domain (it commutes with
  the irfft): 11 scalar_tensor_tensor taps with per-partition dw scalars.
  Glide-reflection boundary rows enter via per-core sign-row inputs that
  are zero on interior cores, keeping one uniform SPMD program.
- Single irfft matmul per output row computes y + z directly.
"""

import math
import os
from contextlib import ExitStack

import numpy as np
import ml_dtypes

import concourse.bass as bass
from concourse import bacc
import concourse.mybir as mybir
from concourse import bass_utils
from concourse.tile import TileContext
from concourse.masks import make_identity
from concourse.bass import ds, ts

F32 = mybir.dt.float32
BF16 = mybir.dt.bfloat16
AF = mybir.ActivationFunctionType
OP = mybir.AluOpType

B, C, H, W = 2, 64, 360, 720
KF = W // 2 + 1          # 361
K2 = 2 * KF              # 722 (Re | Im)
BC = B * C               # 128
NCORES = 8
HB = H // NCORES         # 45 own rows per core
HALO = 5
NH = HB + 2 * HALO       # 55 spectral rows per core
NHG = NH + 10            # + 5 pre-glide + 5 post-glide tmp rows
WP = 768                 # W padded to 6*128 for chunked matmuls
KP = 768                 # spectrum padded to 6*128
NCHUNK = 6
KE = 362                 # even-width bf16 temporaries (DVE 2x mode)

_CACHE = {}


def _dft_matrices():
    n = np.arange(W)[:, None].astype(np.float64)
    k = np.arange(KF)[None, :].astype(np.float64)
    ang = 2.0 * np.pi * n * k / W
    s = 1.0 / math.sqrt(W)
    Fre = np.cos(ang) * s
    Fim = -np.sin(ang) * s
    wk = np.full(KF, 2.0)
    wk[0] = 1.0
    wk[KF - 1] = 1.0
    Ire = (np.cos(ang) * wk * s).T      # [KF, W]
    Iim = (-np.sin(ang) * wk * s).T     # [KF, W]
    # fwd matmul rhs chunks: [NCHUNK, 128, K2]; rows are w within chunk
    fdft = np.zeros((NCHUNK, 128, K2), np.float32)
    for t in range(NCHUNK):
        w0 = 128 * t
        w1 = min(W, w0 + 128)
        fdft[t, : w1 - w0, :KF] = Fre[w0:w1]
        fdft[t, : w1 - w0, KF:] = Fim[w0:w1]
    # inv matmul rhs chunks: [NCHUNK, 128, W]; rows are spectrum comps
    # spectrum layout: comp c in [0, 722): c < 361 -> Re[k=c], else Im[k=c-361]
    minv = np.zeros((NCHUNK, 128, W), np.float32)
    icat = np.concatenate([Ire, Iim], 0)   # [722, W]
    for t in range(NCHUNK):
        c0 = 128 * t
        c1 = min(K2, c0 + 128)
        minv[t, : c1 - c0, :] = icat[c0:c1]
    return fdft, minv


def build_program(fast: bool):
    assert fast, "device program implements the fast (eval-buffer) path"
    nc = bacc.Bacc("TRN2", target_bir_lowering=False, debug=False, num_devices=NCORES)

    # ---- dram I/O ----
    xT_d = nc.dram_tensor("xT", [NH, NCHUNK, 128, 128], BF16, kind="ExternalInput")
    fdft_d = nc.dram_tensor("fdft", [NCHUNK, 128, K2], BF16, kind="ExternalInput")
    minv_d = nc.dram_tensor("minv", [NCHUNK, 128, W], BF16, kind="ExternalInput")
    istd_d = nc.dram_tensor("istd", [NH, KF], F32, kind="ExternalInput")
    glum_d = nc.dram_tensor("glum", [NH, 128, KF], F32, kind="ExternalInput")
    glup_d = nc.dram_tensor("glup", [NH, 128, KF], F32, kind="ExternalInput")
    w1m_d = nc.dram_tensor("w1m", [4, 128, 128], BF16, kind="ExternalInput")
    w2m_d = nc.dram_tensor("w2m", [4, 128, 128], BF16, kind="ExternalInput")
    brelu_d = nc.dram_tensor("brelu", [128, 1], F32, kind="ExternalInput")
    dwd_d = nc.dram_tensor("dwdiag", [11, 128, 128], BF16, kind="ExternalInput")
    sgnpre_d = nc.dram_tensor("sgnpre", [128, KP], BF16, kind="ExternalInput")
    sgnpost_d = nc.dram_tensor("sgnpost", [128, KP], BF16, kind="ExternalInput")
    if not fast:
        mtr_d = nc.dram_tensor("mtr", [NH, KF], F32, kind="ExternalInput")
        mti_d = nc.dram_tensor("mti", [NH, KF], F32, kind="ExternalInput")
        stdm_d = nc.dram_tensor("stdm", [NH, KF], F32, kind="ExternalInput")
        s1_d = nc.dram_tensor("s1", [128, 4], F32, kind="ExternalInput")  # -S1r, S1i, -S1i, -S1r? see below
        b1_d = nc.dram_tensor("b1", [128, 2], F32, kind="ExternalInput")
    y_d = nc.dram_tensor("y", [HB, 128, W], F32, kind="ExternalOutput")

    with TileContext(nc) as tc, ExitStack() as ctx:
        consts = ctx.enter_context(tc.tile_pool(name="consts", bufs=1))
        bigpool = ctx.enter_context(tc.tile_pool(name="big", bufs=1))
        xpool = ctx.enter_context(tc.tile_pool(name="xp", bufs=3))
        glupool = ctx.enter_context(tc.tile_pool(name="glu", bufs=2))
        bpool = ctx.enter_context(tc.tile_pool(name="bcast", bufs=2))
        cpool = ctx.enter_context(tc.tile_pool(name="chain", bufs=2))
        xxpool = ctx.enter_context(tc.tile_pool(name="xx", bufs=2))
        spool = ctx.enter_context(tc.tile_pool(name="sacc", bufs=2))
        opool = ctx.enter_context(tc.tile_pool(name="out", bufs=2))
        psX = ctx.enter_context(tc.tile_pool(name="psX", bufs=1, space="PSUM"))
        psG = ctx.enter_context(tc.tile_pool(name="psG", bufs=2, space="PSUM"))
        psOwn = ctx.enter_context(tc.tile_pool(name="psOwn", bufs=1, space="PSUM"))

        # ---- constants in SBUF ----
        ident = consts.tile([128, 128], BF16)
        make_identity(nc, ident)
        fdft_sb = consts.tile([128, NCHUNK, K2], BF16)
        nc.sync.dma_start(fdft_sb, fdft_d[:, :, :].rearrange("t p k -> p t k"))
        minv_sb = consts.tile([128, NCHUNK, W], BF16)
        nc.sync.dma_start(minv_sb, minv_d[:, :, :].rearrange("t p k -> p t k"))
        w1m = consts.tile([128, 4, 128], BF16)
        nc.sync.dma_start(w1m, w1m_d[:, :, :].rearrange("i p m -> p i m"))
        w2m = consts.tile([128, 4, 128], BF16)
        nc.sync.dma_start(w2m, w2m_d[:, :, :].rearrange("i p m -> p i m"))
        brelu = consts.tile([128, 1], F32)
        nc.sync.dma_start(brelu, brelu_d[:, :])
        dwdiag = consts.tile([128, 11, 128], BF16)
        nc.sync.dma_start(dwdiag, dwd_d[:, :, :].rearrange("u p m -> p u m"))
        sgnpre = consts.tile([128, KP], BF16)
        nc.sync.dma_start(sgnpre, sgnpre_d[:, :])
        sgnpost = consts.tile([128, KP], BF16)
        nc.sync.dma_start(sgnpost, sgnpost_d[:, :])
        epsb = consts.tile([128, 1], F32)
        nc.vector.memset(epsb, 1e-30)
        halfpib = consts.tile([128, 1], F32)
        nc.vector.memset(halfpib, math.pi / 2)
        if not fast:
            s1 = consts.tile([128, 4], F32)
            nc.sync.dma_start(s1, s1_d[:, :])
            b1 = consts.tile([128, 2], F32)
            nc.sync.dma_start(b1, b1_d[:, :])

        # Y slab: rows 0..NH-1 spectrum rows, NH..NH+4 pre-glide tmp,
        # NH+5..NH+9 post-glide tmp.  bf16 [128, NHG, KP].
        yslab = bigpool.tile([128, NHG, KP], BF16)
        nc.gpsimd.memset(yslab[:, :, K2:KP], 0.0)

        def dma_bcast(out_tile, dram_t, s):
            a = dram_t[s:s + 1, :]
            bc = bass.AP(tensor=a.tensor, offset=a.offset,
                         ap=[[0, 128], a.ap[-1]])
            nc.sync.dma_start(out_tile, bc)

        def cmul_mm(ps_re, ps_im, wm, xr, xi):
            """ps_re/im [128,361] += complex W @ (xr + i xi).
            wm: [128, 4, 128] tiles: 0=WrT, 1=negWiT, 2=WiT, 3=WrT(copy)."""
            nc.tensor.matmul(ps_re, wm[:, 0, :], xr, start=True, stop=False)
            nc.tensor.matmul(ps_re, wm[:, 1, :], xi, start=False, stop=True)
            nc.tensor.matmul(ps_im, wm[:, 2, :], xr, start=True, stop=False)
            nc.tensor.matmul(ps_im, wm[:, 3, :], xi, start=False, stop=True)

        # ================= spectral rows (paired, phase-batched ACT) ======
        def spectral_pair(slist):
            px, gm_t, gp_t = {}, {}, {}
            for s in slist:
                xT = xpool.tile([128, NCHUNK, 128], BF16)
                nc.sync.dma_start(xT, xT_d[s].rearrange("t p m -> p t m"))
                glum = glupool.tile([128, KF], F32, tag="glum")
                nc.sync.dma_start(glum, glum_d[s])
                glup = glupool.tile([128, KF], F32, tag="glup")
                nc.sync.dma_start(glup, glup_d[s])
                gm_t[s], gp_t[s] = glum, glup

                ps_xre = psX.tile([128, KF], F32, tag="psxr")
                ps_xim = psX.tile([128, KF], F32, tag="psxi")
                for t in range(NCHUNK):
                    nc.tensor.matmul(ps_xre, xT[:, t, :], fdft_sb[:, t, 0:KF],
                                     start=(t == 0), stop=(t == NCHUNK - 1))
                    nc.tensor.matmul(ps_xim, xT[:, t, :], fdft_sb[:, t, KF:K2],
                                     start=(t == 0), stop=(t == NCHUNK - 1))
                istd_b = bpool.tile([128, KF], F32, tag="istdb")
                dma_bcast(istd_b, istd_d, s)
                xr = xxpool.tile([128, KF], BF16, tag="xr")
                nc.vector.tensor_tensor(xr, ps_xre, istd_b, OP.mult)
                xi = xxpool.tile([128, KF], BF16, tag="xi")
                nc.vector.tensor_tensor(xi, ps_xim, istd_b, OP.mult)
                px[s] = (xr, xi)

            g1, r2t = {}, {}
            for s in slist:
                ps_g1r = psG.tile([128, KF], F32, tag="psgr")
                ps_g1i = psG.tile([128, KF], F32, tag="psgi")
                cmul_mm(ps_g1r, ps_g1i, w1m, px[s][0], px[s][1])
                g1[s] = (ps_g1r, ps_g1i)
            for s in slist:
                sq_r = cpool.tile([128, KF], F32, tag="sqr")
                nc.scalar.activation(sq_r, g1[s][0], AF.Square)
                sq_i = cpool.tile([128, KF], F32, tag="sqi")
                nc.scalar.activation(sq_i, g1[s][1], AF.Square)
                r2 = cpool.tile([128, KF], F32, tag="r2")
                nc.gpsimd.tensor_tensor(r2, sq_r, sq_i, OP.add)
                r2t[s] = r2
            r1t = {}
            for s in slist:
                r1 = cpool.tile([128, KF], F32, tag="r1")
                nc.scalar.activation(r1, r2t[s], AF.Sqrt, bias=epsb[:, 0:1])
                r1t[s] = r1
            sc1t = {}
            for s in slist:
                e1 = cpool.tile([128, KF], F32, tag="g1")
                nc.scalar.activation(e1, r1t[s], AF.Erf, scale=1.0 / math.sqrt(2.0))
                sc1 = cpool.tile([128, KF], F32, tag="sc1")
                nc.vector.tensor_scalar(sc1, e1, 0.5, 0.5, OP.mult, OP.add)
                sc1t[s] = sc1
            h2m = {}
            for s in slist:
                h2mr = cpool.tile([128, KF], BF16, tag="h2mr")
                nc.vector.tensor_tensor(h2mr, g1[s][0], sc1t[s], OP.mult)
                h2mi = cpool.tile([128, KF], BF16, tag="h2mi")
                nc.vector.tensor_tensor(h2mi, g1[s][1], sc1t[s], OP.mult)
                h2m[s] = (h2mr, h2mi)
            g2 = {}
            for s in slist:
                ps_g2r = psG.tile([128, KF], F32, tag="psgr")
                ps_g2i = psG.tile([128, KF], F32, tag="psgi")
                cmul_mm(ps_g2r, ps_g2i, w2m, h2m[s][0], h2m[s][1])
                g2[s] = (ps_g2r, ps_g2i)
            r2bt = {}
            for s in slist:
                sq2r = cpool.tile([128, KF], F32, tag="sqr")
                nc.scalar.activation(sq2r, g2[s][0], AF.Square)
                sq2i = cpool.tile([128, KF], F32, tag="sqi")
                nc.scalar.activation(sq2i, g2[s][1], AF.Square)
                r2b = cpool.tile([128, KF], F32, tag="r2")
                nc.gpsimd.tensor_tensor(r2b, sq2r, sq2i, OP.add)
                r2bt[s] = r2b
            rbt = {}
            for s in slist:
                rb = cpool.tile([128, KF], F32, tag="r1")
                nc.scalar.activation(rb, r2bt[s], AF.Sqrt, bias=epsb[:, 0:1])
                rbt[s] = rb
            tht = {}
            for s in slist:
                tg = cpool.tile([128, KF], F32, tag="tg")
                nc.vector.tensor_tensor(tg, rbt[s], gm_t[s], OP.add)
                th = cpool.tile([128, KF], F32, tag="g1")
                nc.scalar.activation(th, tg, AF.Tanh, scale=0.5)
                tht[s] = th
            trig = {}
            for s in slist:
                cosp = cpool.tile([128, KF], F32, tag="cosp")
                nc.scalar.activation(cosp, gp_t[s], AF.Sin, bias=halfpib[:, 0:1])
                sinp = cpool.tile([128, KF], F32, tag="sinp")
                nc.scalar.activation(sinp, gp_t[s], AF.Sin)
                trig[s] = (cosp, sinp)
            for s in slist:
                sg = cpool.tile([128, KF], F32, tag="sg")
                nc.vector.tensor_scalar(sg, tht[s], 0.5, 0.5, OP.mult, OP.add)
                rbinv = cpool.tile([128, KF], F32, tag="rbinv")
                nc.vector.reciprocal_approx_fast(rbinv, rbt[s])
                rho = cpool.tile([128, KF], F32, tag="rho")
                nc.gpsimd.tensor_tensor(rho, sg, rbinv, OP.mult)
                wre = cpool.tile([128, KF], F32, tag="wre")
                nc.gpsimd.tensor_tensor(wre, rho, trig[s][0], OP.mult)
                wim = cpool.tile([128, KF], F32, tag="wim")
                nc.gpsimd.tensor_tensor(wim, rho, trig[s][1], OP.mult)
                ps_g2r, ps_g2i = g2[s]
                t_a = cpool.tile([128, KF], BF16, tag="t_a")
                nc.vector.tensor_tensor(t_a, ps_g2r, wre, OP.mult)
                t_b = cpool.tile([128, KF], BF16, tag="t_b")
                nc.vector.tensor_tensor(t_b, ps_g2i, wim, OP.mult)
                t_c = cpool.tile([128, KF], BF16, tag="t_c")
                nc.vector.tensor_tensor(t_c, ps_g2r, wim, OP.mult)
                t_d = cpool.tile([128, KF], BF16, tag="t_d")
                nc.vector.tensor_tensor(t_d, ps_g2i, wre, OP.mult)
                ur = cpool.tile([128, KF], BF16, tag="ur")
                nc.gpsimd.tensor_tensor(ur, t_a, t_b, OP.subtract)
                ui = cpool.tile([128, KF], BF16, tag="ui")
                nc.gpsimd.tensor_tensor(ui, t_c, t_d, OP.add)
                xr, xi = px[s]
                v_a = cpool.tile([128, KF], BF16, tag="t_a")
                nc.vector.tensor_tensor(v_a, xr, ur, OP.mult)
                v_b = cpool.tile([128, KF], BF16, tag="t_b")
                nc.vector.tensor_tensor(v_b, xi, ui, OP.mult)
                v_c = cpool.tile([128, KF], BF16, tag="t_c")
                nc.vector.tensor_tensor(v_c, xr, ui, OP.mult)
                v_d = cpool.tile([128, KF], BF16, tag="t_d")
                nc.vector.tensor_tensor(v_d, xi, ur, OP.mult)
                nc.gpsimd.tensor_tensor(yslab[:, s, 0:KF], v_a, v_b, OP.subtract)
                nc.gpsimd.tensor_tensor(yslab[:, s, KF:K2], v_c, v_d, OP.add)

        for s0 in range(0, NH, 2):
            spectral_pair(list(range(s0, min(s0 + 2, NH))))

        # ============ glide tmp rows ============
        # pre: tmp[p] = sgnpre * Y[slab row 9-p],  p = 0..4  -> row NH+p
        # post: tmp[q] = sgnpost * Y[slab row 49-q], q = 0..4 -> row NH+5+q
        for p in range(5):
            nc.vector.tensor_tensor(yslab[:, NH + p, :],
                                     yslab[:, 9 - p, :], sgnpre, OP.mult)
        for q in range(5):
            nc.vector.tensor_tensor(yslab[:, NH + 5 + q, :],
                                     yslab[:, 49 - q, :], sgnpost, OP.mult)

        # ============ own rows: conv (PE diag matmuls) + irfft ============
        for h in range(HB):
            # taps: (lhsT weight tile, source slab row)
            taps = [(ident, h + HALO)]
            taps += [(dwdiag[:, u, :], h + u) for u in range(11)]
            for u in range(11):
                if h + u <= 4:
                    taps.append((dwdiag[:, u, :], NH + h + u))
                if h + u >= 50:
                    taps.append((dwdiag[:, u, :], NH + 5 + (h + u - 50)))
            ps_cr = psOwn.tile([128, KF], F32, tag="cr")
            ps_ci = psOwn.tile([128, KF], F32, tag="ci")
            last = len(taps) - 1
            for i, (w_t, row) in enumerate(taps):
                nc.tensor.matmul(ps_cr, w_t, yslab[:, row, 0:KF],
                                 start=(i == 0), stop=(i == last))
                nc.tensor.matmul(ps_ci, w_t, yslab[:, row, KF:K2],
                                 start=(i == 0), stop=(i == last))
            scnv = spool.tile([128, KP], BF16, tag="scnv")
            nc.scalar.copy(scnv[:, 0:KF], ps_cr)
            nc.vector.tensor_copy(scnv[:, KF:K2], ps_ci)
            nc.vector.memset(scnv[:, K2:KP], 0.0)

            # corner turn: 6 transposes [128,128] -> one psum tile [128, 768]
            ps_t = psOwn.tile([128, KP], BF16, tag="cr")
            for t in range(NCHUNK):
                nc.tensor.transpose(ps_t[:, ts(t, 128)], scnv[:, ts(t, 128)], ident)
            sT = spool.tile([128, KP], BF16, tag="sT")
            nc.vector.tensor_copy(sT, ps_t)

            # irfft: y = S^T . Minv  (+ accumulate over 6 chunks)
            ps_ya = psOwn.tile([128, 360], F32, tag="cr")
            ps_yb = psOwn.tile([128, 360], F32, tag="ci")
            for t in range(NCHUNK):
                nc.tensor.matmul(ps_ya, sT[:, ts(t, 128)], minv_sb[:, t, 0:360],
                                 start=(t == 0), stop=(t == NCHUNK - 1))
                nc.tensor.matmul(ps_yb, sT[:, ts(t, 128)], minv_sb[:, t, 360:720],
                                 start=(t == 0), stop=(t == NCHUNK - 1))
            yrow = opool.tile([128, W], F32)
            nc.scalar.copy(yrow[:, 0:360], ps_ya)
            nc.scalar.copy(yrow[:, 360:720], ps_yb)
            nc.sync.dma_start(y_d[h], yrow)

    nc.finalize()
    return nc


def _host_prep(inputs, fast):
    """Build per-core input maps (host-side numpy only)."""
    x = np.ascontiguousarray(np.asarray(inputs["x"], np.float32).reshape(BC, H, W))
    rows = np.asarray(inputs["rows"])
    cols = np.asarray(inputs["cols"])

    mask = np.zeros((H, KF), np.float32)
    mask[rows, cols] = 1.0
    std_d = np.ones((H, KF), np.float32)
    std_d[rows, cols] = np.asarray(inputs["sr_std"], np.float32)
    istd_d = mask / (1e-12 + std_d)
    gm_d = np.zeros((C, H, KF), np.float32)
    gm_d[:, rows, cols] = np.asarray(inputs["glu_mags"], np.float32)
    gp_d = np.zeros((C, H, KF), np.float32)
    gp_d[:, rows, cols] = np.asarray(inputs["glu_phases"], np.float32)

    if not fast:
        mr_d = np.zeros((H, KF), np.float32)
        mr_d[rows, cols] = np.asarray(inputs["sr_mean_r"], np.float32)
        mi_d = np.zeros((H, KF), np.float32)
        mi_d[rows, cols] = np.asarray(inputs["sr_mean_i"], np.float32)
        mtr_d = mr_d * istd_d
        mti_d = mi_d * istd_d
        stdm_d = mask * (1e-12 + std_d)

    W1r = np.asarray(inputs["w1_r"], np.float32)
    W1i = np.asarray(inputs["w1_i"], np.float32)
    magr = np.asarray(inputs["sr_mags_r"], np.float32)
    magi = np.asarray(inputs["sr_mags_i"], np.float32)
    W1pr = W1r * magr[None, :] - W1i * magi[None, :]
    W1pi = W1r * magi[None, :] + W1i * magr[None, :]
    W2r = np.asarray(inputs["w2_r"], np.float32)
    W2i = np.asarray(inputs["w2_i"], np.float32)

    def bd(M):  # [64,64] -> block-diag [128,128] of M^T (lhsT layout)
        out = np.zeros((128, 128), np.float32)
        out[:C, :C] = M.T
        out[C:, C:] = M.T
        return out

    w1m = np.stack([bd(W1pr), bd(-W1pi), bd(W1pi), bd(W1pr)])
    w2m = np.stack([bd(W2r), bd(-W2i), bd(W2i), bd(W2r)])

    fdft, minv = _dft_matrices()
    bf = ml_dtypes.bfloat16

    sgnk = ((-1.0) ** np.arange(KF)).astype(np.float32)
    sgnrow = np.zeros(KP, np.float32)
    sgnrow[0:KF] = sgnk
    sgnrow[KF:K2] = sgnk

    dwt = np.asarray(inputs["dw_weight"], np.float32)
    dw_bc = np.tile(dwt, (2, 1)).astype(np.float32)          # [128, 11]
    dwdiag = np.zeros((11, 128, 128), np.float32)
    for u in range(11):
        np.fill_diagonal(dwdiag[u], dw_bc[:, u])
    brelu = np.full((128, 1), float(np.asarray(inputs["b_relu"])), np.float32)

    common = dict(
        fdft=fdft.astype(bf),
        minv=minv.astype(bf),
        w1m=w1m.astype(bf),
        w2m=w2m.astype(bf),
        brelu=brelu,
        dwdiag=dwdiag.astype(bf),
    )
    if not fast:
        S1 = (W1pr + 1j * W1pi).sum(1)
        bias = np.asarray(inputs["sr_bias_r"], np.float32) + 1j * np.asarray(
            inputs["sr_bias_i"], np.float32)
        B1 = (W1r + 1j * W1i) @ bias
        s1 = np.stack([
            np.tile(-S1.real, 2), np.tile(S1.imag, 2),
            np.tile(-S1.imag, 2), np.tile(-S1.real, 2)], 1).astype(np.float32)
        b1 = np.stack([np.tile(B1.real, 2), np.tile(B1.imag, 2)], 1).astype(np.float32)
        common.update(s1=s1, b1=b1)

    in_maps = []
    for r in range(NCORES):
        h0 = HB * r - HALO
        gidx = np.arange(h0, h0 + NH)
        valid = (gidx >= 0) & (gidx < H)
        gv = np.clip(gidx, 0, H - 1)

        xs = np.zeros((NH, BC, WP), np.float32)
        xs[valid, :, :W] = x[:, gv[valid], :].transpose(1, 0, 2)
        # transpose w <-> bc per chunk: [NH, 6, 128(w), 128(bc)]
        xT = np.ascontiguousarray(
            xs.reshape(NH, BC, NCHUNK, 128).transpose(0, 2, 3, 1)).astype(bf)

        def rowsel(d2):   # [H, KF] -> [1, NH*KF] with invalid rows zeroed
            out = np.zeros((NH, KF), np.float32)
            out[valid] = d2[gv[valid]]
            return out

        def glusel(d3):   # [C, H, KF] -> [NH, 128, KF]
            out = np.zeros((NH, BC, KF), np.float32)
            sel = d3[:, gv[valid], :].transpose(1, 0, 2)   # [nvalid, C, KF]
            out[valid] = np.concatenate([sel, sel], 1)
            return out

        m = dict(common)
        m.update(
            xT=xT,
            istd=rowsel(istd_d),
            glum=glusel(gm_d),
            glup=glusel(gp_d),
            sgnpre=np.broadcast_to(
                sgnrow * (1.0 if r == 0 else 0.0), (128, KP)).astype(bf).copy(),
            sgnpost=np.broadcast_to(
                sgnrow * (1.0 if r == NCORES - 1 else 0.0), (128, KP)).astype(bf).copy(),
        )
        if not fast:
            m.update(mtr=rowsel(mtr_d), mti=rowsel(mti_d), stdm=rowsel(stdm_d))
        in_maps.append(m)
    return in_maps


def kernel(**inputs):
    fast = bool(
        np.all(np.asarray(inputs["sr_mean_r"]) == 0)
        and np.all(np.asarray(inputs["sr_mean_i"]) == 0)
        and np.all(np.asarray(inputs["sr_bias_r"]) == 0)
        and np.all(np.asarray(inputs["sr_bias_i"]) == 0)
        and np.all(np.asarray(inputs["sr_std"]) == 1)
        and float(np.asarray(inputs["b_relu"])) == 0.0
    )
    if not fast:
        return _numpy_fallback(inputs)
    if ("prog", fast) not in _CACHE:
        _CACHE[("prog", fast)] = build_program(fast)
    nc = _CACHE[("prog", fast)]

    in_maps = _host_prep(inputs, fast)
    res = bass_utils.run_bass_kernel_spmd(
        nc, in_maps, core_ids=list(range(NCORES)),
        trace=bool(int(os.environ.get("KTRACE", "0"))),
    )
    kernel.last_results = res

    out = np.zeros((BC, H, W), np.float32)
    for r in range(NCORES):
        y = res.results[r]["y"]            # [HB, 128, W]
        out[:, HB * r:HB * (r + 1), :] = y.transpose(1, 0, 2)
    return out.reshape(B, C, H, W).astype(np.float32)


def _numpy_fallback(inputs):
    """Exact-math host fallback for the general (non-eval-buffer) case."""
    from numpy import fft as _fft
    x = np.asarray(inputs["x"], np.float32)
    rows = np.asarray(inputs["rows"]); cols = np.asarray(inputs["cols"])
    xf = _fft.rfft(x, axis=-1, norm="ortho")
    xm = xf[:, :, rows, cols]
    mean = (np.asarray(inputs["sr_mean_r"]) + 1j * np.asarray(inputs["sr_mean_i"]))[None, None]
    h = (xm - mean) / (1e-12 + np.asarray(inputs["sr_std"])[None, None])
    h = h * (np.asarray(inputs["sr_mags_r"]) + 1j * np.asarray(inputs["sr_mags_i"]))[None, :, None] \
        + (np.asarray(inputs["sr_bias_r"]) + 1j * np.asarray(inputs["sr_bias_i"]))[None, :, None]
    h = np.einsum("oi,bit->bot", np.asarray(inputs["w1_r"]) + 1j * np.asarray(inputs["w1_i"]), h)
    from scipy.special import erf as _erf
    r = np.abs(h) + float(np.asarray(inputs["b_relu"]))
    g = 0.5 * r * (1.0 + _erf(r / np.sqrt(2.0)))
    h = g * np.exp(1j * np.angle(h))
    h = np.einsum("oi,bit->bot", np.asarray(inputs["w2_r"]) + 1j * np.asarray(inputs["w2_i"]), h)
    gate = 1.0 / (1.0 + np.exp(-(np.abs(h) + np.asarray(inputs["glu_mags"])[None]))) \
        * np.exp(1j * (np.angle(h) + np.asarray(inputs["glu_phases"])[None]))
    xm = xm * gate
    xf2 = np.zeros_like(xf)
    xf2[:, :, rows, cols] = xm
    y = _fft.irfft(xf2, n=W, axis=-1, norm="ortho").astype(np.float32)
    gr = np.roll(np.flip(y, axis=2), W // 2, axis=3)
    ypad = np.concatenate([gr[:, :, -5:], y, gr[:, :, :5]], axis=2)
    dw = np.asarray(inputs["dw_weight"])
    z = np.zeros_like(y)
    for u in range(11):
        z += dw[None, :, u, None, None] * ypad[:, :, u:u + H, :]
    return (y + z).astype(np.float32)


# revision 27
# speedup vs baseline: 1.0105x; 1.0105x over previous
"""Trainium2 Bass kernel for DSConvSpectral.

Contract: kernel(**inputs) takes FULL unsharded inputs (as produced by
setup_inputs()) and returns the FULL [2,64,360,720] float32 output.

Strategy (8 NeuronCores, SPMD):
- Shard latitude H into 8 contiguous blocks of 45 rows; each core also
  computes the 5-row halo on each side (duplicated work, no collectives).
- Per spectral row: dense rfft along W as bf16 matmuls against precomputed
  DFT matrices (x is host-pre-transposed so the contraction dim lands on
  partitions); pointwise chain (SpectralRescale folded into W1 on host,
  mod_relu, W2, complex GLU gate) split across DVE/ACT/GPSIMD; result
  spectrum rows stored in an SBUF-resident bf16 slab.
- Depthwise (11,1) H-conv done in the frequency domain (it commutes with
  the irfft): 11 scalar_tensor_tensor taps with per-partition dw scalars.
  Glide-reflection boundary rows enter via per-core sign-row inputs that
  are zero on interior cores, keeping one uniform SPMD program.
- Single irfft matmul per output row computes y + z directly.
"""

import math
import os
from contextlib import ExitStack

import numpy as np
import ml_dtypes

import concourse.bass as bass
from concourse import bacc
import concourse.mybir as mybir
from concourse import bass_utils
from concourse.tile import TileContext
from concourse.masks import make_identity
from concourse.bass import ds, ts

F32 = mybir.dt.float32
BF16 = mybir.dt.bfloat16
AF = mybir.ActivationFunctionType
OP = mybir.AluOpType

B, C, H, W = 2, 64, 360, 720
KF = W // 2 + 1          # 361
K2 = 2 * KF              # 722 (Re | Im)
BC = B * C               # 128
NCORES = 8
HB = H // NCORES         # 45 own rows per core
HALO = 5
NH = HB + 2 * HALO       # 55 spectral rows per core
NHG = NH + 10            # + 5 pre-glide + 5 post-glide tmp rows
WP = 768                 # W padded to 6*128 for chunked matmuls
KP = 768                 # spectrum padded to 6*128
NCHUNK = 6
KE = 362                 # even-width bf16 temporaries (DVE 2x mode)

_CACHE = {}


def _dft_matrices():
    n = np.arange(W)[:, None].astype(np.float64)
    k = np.arange(KF)[None, :].astype(np.float64)
    ang = 2.0 * np.pi * n * k / W
    s = 1.0 / math.sqrt(W)
    Fre = np.cos(ang) * s
    Fim = -np.sin(ang) * s
    wk = np.full(KF, 2.0)
    wk[0] = 1.0
    wk[KF - 1] = 1.0
    Ire = (np.cos(ang) * wk * s).T      # [KF, W]
    Iim = (-np.sin(ang) * wk * s).T     # [KF, W]
    # fwd matmul rhs chunks: [NCHUNK, 128, K2]; rows are w within chunk
    fdft = np.zeros((NCHUNK, 128, K2), np.float32)
    for t in range(NCHUNK):
        w0 = 128 * t
        w1 = min(W, w0 + 128)
        fdft[t, : w1 - w0, :KF] = Fre[w0:w1]
        fdft[t, : w1 - w0, KF:] = Fim[w0:w1]
    # inv matmul rhs chunks: [NCHUNK, 128, W]; rows are spectrum comps
    # spectrum layout: comp c in [0, 722): c < 361 -> Re[k=c], else Im[k=c-361]
    minv = np.zeros((NCHUNK, 128, W), np.float32)
    icat = np.concatenate([Ire, Iim], 0)   # [722, W]
    for t in range(NCHUNK):
        c0 = 128 * t
        c1 = min(K2, c0 + 128)
        minv[t, : c1 - c0, :] = icat[c0:c1]
    return fdft, minv


def build_program(fast: bool):
    assert fast, "device program implements the fast (eval-buffer) path"
    nc = bacc.Bacc("TRN2", target_bir_lowering=False, debug=False, num_devices=NCORES)

    # ---- dram I/O ----
    xT_d = nc.dram_tensor("xT", [NH, NCHUNK, 128, 128], BF16, kind="ExternalInput")
    fdft_d = nc.dram_tensor("fdft", [NCHUNK, 128, K2], BF16, kind="ExternalInput")
    minv_d = nc.dram_tensor("minv", [NCHUNK, 128, W], BF16, kind="ExternalInput")
    istd_d = nc.dram_tensor("istd", [NH, KF], F32, kind="ExternalInput")
    glum_d = nc.dram_tensor("glum", [NH, 128, KF], F32, kind="ExternalInput")
    glup_d = nc.dram_tensor("glup", [NH, 128, KF], F32, kind="ExternalInput")
    w1m_d = nc.dram_tensor("w1m", [4, 128, 128], BF16, kind="ExternalInput")
    w2m_d = nc.dram_tensor("w2m", [4, 128, 128], BF16, kind="ExternalInput")
    brelu_d = nc.dram_tensor("brelu", [128, 1], F32, kind="ExternalInput")
    dwd_d = nc.dram_tensor("dwdiag", [11, 128, 128], BF16, kind="ExternalInput")
    sgnpre_d = nc.dram_tensor("sgnpre", [128, KP], BF16, kind="ExternalInput")
    sgnpost_d = nc.dram_tensor("sgnpost", [128, KP], BF16, kind="ExternalInput")
    if not fast:
        mtr_d = nc.dram_tensor("mtr", [NH, KF], F32, kind="ExternalInput")
        mti_d = nc.dram_tensor("mti", [NH, KF], F32, kind="ExternalInput")
        stdm_d = nc.dram_tensor("stdm", [NH, KF], F32, kind="ExternalInput")
        s1_d = nc.dram_tensor("s1", [128, 4], F32, kind="ExternalInput")  # -S1r, S1i, -S1i, -S1r? see below
        b1_d = nc.dram_tensor("b1", [128, 2], F32, kind="ExternalInput")
    y_d = nc.dram_tensor("y", [HB, 128, W], F32, kind="ExternalOutput")

    with TileContext(nc) as tc, ExitStack() as ctx:
        consts = ctx.enter_context(tc.tile_pool(name="consts", bufs=1))
        bigpool = ctx.enter_context(tc.tile_pool(name="big", bufs=1))
        xpool = ctx.enter_context(tc.tile_pool(name="xp", bufs=3))
        glupool = ctx.enter_context(tc.tile_pool(name="glu", bufs=2))
        bpool = ctx.enter_context(tc.tile_pool(name="bcast", bufs=2))
        cpool = ctx.enter_context(tc.tile_pool(name="chain", bufs=2))
        xxpool = ctx.enter_context(tc.tile_pool(name="xx", bufs=2))
        spool = ctx.enter_context(tc.tile_pool(name="sacc", bufs=2))
        opool = ctx.enter_context(tc.tile_pool(name="out", bufs=3))
        psX = ctx.enter_context(tc.tile_pool(name="psX", bufs=1, space="PSUM"))
        psG = ctx.enter_context(tc.tile_pool(name="psG", bufs=2, space="PSUM"))
        psOwn = ctx.enter_context(tc.tile_pool(name="psOwn", bufs=1, space="PSUM"))

        # ---- constants in SBUF ----
        ident = consts.tile([128, 128], BF16)
        make_identity(nc, ident)
        fdft_sb = consts.tile([128, NCHUNK, K2], BF16)
        nc.sync.dma_start(fdft_sb, fdft_d[:, :, :].rearrange("t p k -> p t k"))
        minv_sb = consts.tile([128, NCHUNK, W], BF16)
        nc.sync.dma_start(minv_sb, minv_d[:, :, :].rearrange("t p k -> p t k"))
        w1m = consts.tile([128, 4, 128], BF16)
        nc.sync.dma_start(w1m, w1m_d[:, :, :].rearrange("i p m -> p i m"))
        w2m = consts.tile([128, 4, 128], BF16)
        nc.sync.dma_start(w2m, w2m_d[:, :, :].rearrange("i p m -> p i m"))
        brelu = consts.tile([128, 1], F32)
        nc.sync.dma_start(brelu, brelu_d[:, :])
        dwdiag = consts.tile([128, 11, 128], BF16)
        nc.sync.dma_start(dwdiag, dwd_d[:, :, :].rearrange("u p m -> p u m"))
        sgnpre = consts.tile([128, KP], BF16)
        nc.sync.dma_start(sgnpre, sgnpre_d[:, :])
        sgnpost = consts.tile([128, KP], BF16)
        nc.sync.dma_start(sgnpost, sgnpost_d[:, :])
        epsb = consts.tile([128, 1], F32)
        nc.vector.memset(epsb, 1e-30)
        halfpib = consts.tile([128, 1], F32)
        nc.vector.memset(halfpib, math.pi / 2)
        if not fast:
            s1 = consts.tile([128, 4], F32)
            nc.sync.dma_start(s1, s1_d[:, :])
            b1 = consts.tile([128, 2], F32)
            nc.sync.dma_start(b1, b1_d[:, :])

        # Y slab: rows 0..NH-1 spectrum rows, NH..NH+4 pre-glide tmp,
        # NH+5..NH+9 post-glide tmp.  bf16 [128, NHG, KP].
        yslab = bigpool.tile([128, NHG, K2], BF16)

        def dma_bcast(out_tile, dram_t, s):
            a = dram_t[s:s + 1, :]
            bc = bass.AP(tensor=a.tensor, offset=a.offset,
                         ap=[[0, 128], a.ap[-1]])
            nc.sync.dma_start(out_tile, bc)

        def cmul_mm(ps_re, ps_im, wm, xr, xi):
            """ps_re/im [128,361] += complex W @ (xr + i xi).
            wm: [128, 4, 128] tiles: 0=WrT, 1=negWiT, 2=WiT, 3=WrT(copy)."""
            nc.tensor.matmul(ps_re, wm[:, 0, :], xr, start=True, stop=False)
            nc.tensor.matmul(ps_re, wm[:, 1, :], xi, start=False, stop=True)
            nc.tensor.matmul(ps_im, wm[:, 2, :], xr, start=True, stop=False)
            nc.tensor.matmul(ps_im, wm[:, 3, :], xi, start=False, stop=True)

        # ================= spectral rows (paired, phase-batched ACT) ======
        def spectral_pair(slist):
            px, gm_t, gp_t = {}, {}, {}
            for s in slist:
                xT = xpool.tile([128, NCHUNK, 128], BF16)
                nc.sync.dma_start(xT, xT_d[s].rearrange("t p m -> p t m"))
                glum = glupool.tile([128, KF], F32, tag="glum")
                nc.sync.dma_start(glum, glum_d[s])
                glup = glupool.tile([128, KF], F32, tag="glup")
                nc.sync.dma_start(glup, glup_d[s])
                gm_t[s], gp_t[s] = glum, glup

                ps_xre = psX.tile([128, KF], F32, tag="psxr")
                ps_xim = psX.tile([128, KF], F32, tag="psxi")
                for t in range(NCHUNK):
                    nc.tensor.matmul(ps_xre, xT[:, t, :], fdft_sb[:, t, 0:KF],
                                     start=(t == 0), stop=(t == NCHUNK - 1))
                    nc.tensor.matmul(ps_xim, xT[:, t, :], fdft_sb[:, t, KF:K2],
                                     start=(t == 0), stop=(t == NCHUNK - 1))
                istd_b = bpool.tile([128, KF], F32, tag="istdb")
                dma_bcast(istd_b, istd_d, s)
                xr = xxpool.tile([128, KF], BF16, tag="xr")
                nc.vector.tensor_tensor(xr, ps_xre, istd_b, OP.mult)
                xi = xxpool.tile([128, KF], BF16, tag="xi")
                nc.vector.tensor_tensor(xi, ps_xim, istd_b, OP.mult)
                px[s] = (xr, xi)

            g1, r2t = {}, {}
            for s in slist:
                ps_g1r = psG.tile([128, KF], F32, tag="psgr")
                ps_g1i = psG.tile([128, KF], F32, tag="psgi")
                cmul_mm(ps_g1r, ps_g1i, w1m, px[s][0], px[s][1])
                g1[s] = (ps_g1r, ps_g1i)
            for s in slist:
                sq_r = cpool.tile([128, KF], F32, tag="sqr")
                nc.scalar.activation(sq_r, g1[s][0], AF.Square)
                sq_i = cpool.tile([128, KF], F32, tag="sqi")
                nc.scalar.activation(sq_i, g1[s][1], AF.Square)
                r2 = cpool.tile([128, KF], F32, tag="r2")
                nc.gpsimd.tensor_tensor(r2, sq_r, sq_i, OP.add)
                r2t[s] = r2
            r1t = {}
            for s in slist:
                r1 = cpool.tile([128, KF], F32, tag="r1")
                nc.scalar.activation(r1, r2t[s], AF.Sqrt, bias=epsb[:, 0:1])
                r1t[s] = r1
            sc1t = {}
            for s in slist:
                e1 = cpool.tile([128, KF], F32, tag="g1")
                nc.scalar.activation(e1, r1t[s], AF.Erf, scale=1.0 / math.sqrt(2.0))
                sc1 = cpool.tile([128, KF], F32, tag="sc1")
                nc.vector.tensor_scalar(sc1, e1, 0.5, 0.5, OP.mult, OP.add)
                sc1t[s] = sc1
            h2m = {}
            for s in slist:
                h2mr = cpool.tile([128, KF], BF16, tag="h2mr")
                nc.vector.tensor_tensor(h2mr, g1[s][0], sc1t[s], OP.mult)
                h2mi = cpool.tile([128, KF], BF16, tag="h2mi")
                nc.vector.tensor_tensor(h2mi, g1[s][1], sc1t[s], OP.mult)
                h2m[s] = (h2mr, h2mi)
            g2 = {}
            for s in slist:
                ps_g2r = psG.tile([128, KF], F32, tag="psgr")
                ps_g2i = psG.tile([128, KF], F32, tag="psgi")
                cmul_mm(ps_g2r, ps_g2i, w2m, h2m[s][0], h2m[s][1])
                g2[s] = (ps_g2r, ps_g2i)
            r2bt = {}
            for s in slist:
                sq2r = cpool.tile([128, KF], F32, tag="sqr")
                nc.scalar.activation(sq2r, g2[s][0], AF.Square)
                sq2i = cpool.tile([128, KF], F32, tag="sqi")
                nc.scalar.activation(sq2i, g2[s][1], AF.Square)
                r2b = cpool.tile([128, KF], F32, tag="r2")
                nc.gpsimd.tensor_tensor(r2b, sq2r, sq2i, OP.add)
                r2bt[s] = r2b
            rbt = {}
            for s in slist:
                rb = cpool.tile([128, KF], F32, tag="r1")
                nc.scalar.activation(rb, r2bt[s], AF.Sqrt, bias=epsb[:, 0:1])
                rbt[s] = rb
            tht = {}
            for s in slist:
                tg = cpool.tile([128, KF], F32, tag="tg")
                nc.vector.tensor_tensor(tg, rbt[s], gm_t[s], OP.add)
                th = cpool.tile([128, KF], F32, tag="g1")
                nc.scalar.activation(th, tg, AF.Tanh, scale=0.5)
                tht[s] = th
            trig = {}
            for s in slist:
                cosp = cpool.tile([128, KF], F32, tag="cosp")
                nc.scalar.activation(cosp, gp_t[s], AF.Sin, bias=halfpib[:, 0:1])
                sinp = cpool.tile([128, KF], F32, tag="sinp")
                nc.scalar.activation(sinp, gp_t[s], AF.Sin)
                trig[s] = (cosp, sinp)
            for s in slist:
                sg = cpool.tile([128, KF], F32, tag="sg")
                nc.vector.tensor_scalar(sg, tht[s], 0.5, 0.5, OP.mult, OP.add)
                rbinv = cpool.tile([128, KF], F32, tag="rbinv")
                nc.vector.reciprocal_approx_fast(rbinv, rbt[s])
                rho = cpool.tile([128, KF], F32, tag="rho")
                nc.gpsimd.tensor_tensor(rho, sg, rbinv, OP.mult)
                wre = cpool.tile([128, KF], F32, tag="wre")
                nc.gpsimd.tensor_tensor(wre, rho, trig[s][0], OP.mult)
                wim = cpool.tile([128, KF], F32, tag="wim")
                nc.gpsimd.tensor_tensor(wim, rho, trig[s][1], OP.mult)
                ps_g2r, ps_g2i = g2[s]
                t_a = cpool.tile([128, KF], BF16, tag="t_a")
                nc.vector.tensor_tensor(t_a, ps_g2r, wre, OP.mult)
                t_b = cpool.tile([128, KF], BF16, tag="t_b")
                nc.vector.tensor_tensor(t_b, ps_g2i, wim, OP.mult)
                t_c = cpool.tile([128, KF], BF16, tag="t_c")
                nc.vector.tensor_tensor(t_c, ps_g2r, wim, OP.mult)
                t_d = cpool.tile([128, KF], BF16, tag="t_d")
                nc.vector.tensor_tensor(t_d, ps_g2i, wre, OP.mult)
                ur = cpool.tile([128, KF], BF16, tag="ur")
                nc.gpsimd.tensor_tensor(ur, t_a, t_b, OP.subtract)
                ui = cpool.tile([128, KF], BF16, tag="ui")
                nc.gpsimd.tensor_tensor(ui, t_c, t_d, OP.add)
                xr, xi = px[s]
                v_a = cpool.tile([128, KF], BF16, tag="t_a")
                nc.vector.tensor_tensor(v_a, xr, ur, OP.mult)
                v_b = cpool.tile([128, KF], BF16, tag="t_b")
                nc.vector.tensor_tensor(v_b, xi, ui, OP.mult)
                v_c = cpool.tile([128, KF], BF16, tag="t_c")
                nc.vector.tensor_tensor(v_c, xr, ui, OP.mult)
                v_d = cpool.tile([128, KF], BF16, tag="t_d")
                nc.vector.tensor_tensor(v_d, xi, ur, OP.mult)
                nc.gpsimd.tensor_tensor(yslab[:, s, 0:KF], v_a, v_b, OP.subtract)
                nc.gpsimd.tensor_tensor(yslab[:, s, KF:K2], v_c, v_d, OP.add)

        for s0 in range(0, NH, 2):
            spectral_pair(list(range(s0, min(s0 + 2, NH))))

        # ============ glide tmp rows ============
        # pre: tmp[p] = sgnpre * Y[slab row 9-p],  p = 0..4  -> row NH+p
        # post: tmp[q] = sgnpost * Y[slab row 49-q], q = 0..4 -> row NH+5+q
        for p in range(5):
            nc.vector.tensor_tensor(yslab[:, NH + p, :],
                                     yslab[:, 9 - p, :], sgnpre[:, 0:K2], OP.mult)
        for q in range(5):
            nc.vector.tensor_tensor(yslab[:, NH + 5 + q, :],
                                     yslab[:, 49 - q, :], sgnpost[:, 0:K2], OP.mult)

        # ============ own rows: conv (PE diag matmuls) + irfft ============
        for h in range(HB):
            # taps: (lhsT weight tile, source slab row)
            taps = [(ident, h + HALO)]
            taps += [(dwdiag[:, u, :], h + u) for u in range(11)]
            for u in range(11):
                if h + u <= 4:
                    taps.append((dwdiag[:, u, :], NH + h + u))
                if h + u >= 50:
                    taps.append((dwdiag[:, u, :], NH + 5 + (h + u - 50)))
            ps_cr = psOwn.tile([128, KF], F32, tag="cr")
            ps_ci = psOwn.tile([128, KF], F32, tag="ci")
            last = len(taps) - 1
            for i, (w_t, row) in enumerate(taps):
                nc.tensor.matmul(ps_cr, w_t, yslab[:, row, 0:KF],
                                 start=(i == 0), stop=(i == last))
                nc.tensor.matmul(ps_ci, w_t, yslab[:, row, KF:K2],
                                 start=(i == 0), stop=(i == last))
            scnv = spool.tile([128, KP], BF16, tag="scnv")
            nc.scalar.copy(scnv[:, 0:KF], ps_cr)
            nc.vector.tensor_copy(scnv[:, KF:K2], ps_ci)
            nc.vector.memset(scnv[:, K2:KP], 0.0)

            # corner turn: 6 transposes [128,128] -> one psum tile [128, 768]
            ps_t = psOwn.tile([128, KP], BF16, tag="cr")
            for t in range(NCHUNK):
                nc.tensor.transpose(ps_t[:, ts(t, 128)], scnv[:, ts(t, 128)], ident)
            sT = spool.tile([128, KP], BF16, tag="sT")
            nc.vector.tensor_copy(sT, ps_t)

            # irfft: y = S^T . Minv  (+ accumulate over 6 chunks)
            ps_ya = psOwn.tile([128, 360], F32, tag="cr")
            ps_yb = psOwn.tile([128, 360], F32, tag="ci")
            for t in range(NCHUNK):
                nc.tensor.matmul(ps_ya, sT[:, ts(t, 128)], minv_sb[:, t, 0:360],
                                 start=(t == 0), stop=(t == NCHUNK - 1))
                nc.tensor.matmul(ps_yb, sT[:, ts(t, 128)], minv_sb[:, t, 360:720],
                                 start=(t == 0), stop=(t == NCHUNK - 1))
            yrow = opool.tile([128, W], F32)
            nc.scalar.copy(yrow[:, 0:360], ps_ya)
            nc.scalar.copy(yrow[:, 360:720], ps_yb)
            nc.sync.dma_start(y_d[h], yrow)

    nc.finalize()
    return nc


def _host_prep(inputs, fast):
    """Build per-core input maps (host-side numpy only)."""
    x = np.ascontiguousarray(np.asarray(inputs["x"], np.float32).reshape(BC, H, W))
    rows = np.asarray(inputs["rows"])
    cols = np.asarray(inputs["cols"])

    mask = np.zeros((H, KF), np.float32)
    mask[rows, cols] = 1.0
    std_d = np.ones((H, KF), np.float32)
    std_d[rows, cols] = np.asarray(inputs["sr_std"], np.float32)
    istd_d = mask / (1e-12 + std_d)
    gm_d = np.zeros((C, H, KF), np.float32)
    gm_d[:, rows, cols] = np.asarray(inputs["glu_mags"], np.float32)
    gp_d = np.zeros((C, H, KF), np.float32)
    gp_d[:, rows, cols] = np.asarray(inputs["glu_phases"], np.float32)

    if not fast:
        mr_d = np.zeros((H, KF), np.float32)
        mr_d[rows, cols] = np.asarray(inputs["sr_mean_r"], np.float32)
        mi_d = np.zeros((H, KF), np.float32)
        mi_d[rows, cols] = np.asarray(inputs["sr_mean_i"], np.float32)
        mtr_d = mr_d * istd_d
        mti_d = mi_d * istd_d
        stdm_d = mask * (1e-12 + std_d)

    W1r = np.asarray(inputs["w1_r"], np.float32)
    W1i = np.asarray(inputs["w1_i"], np.float32)
    magr = np.asarray(inputs["sr_mags_r"], np.float32)
    magi = np.asarray(inputs["sr_mags_i"], np.float32)
    W1pr = W1r * magr[None, :] - W1i * magi[None, :]
    W1pi = W1r * magi[None, :] + W1i * magr[None, :]
    W2r = np.asarray(inputs["w2_r"], np.float32)
    W2i = np.asarray(inputs["w2_i"], np.float32)

    def bd(M):  # [64,64] -> block-diag [128,128] of M^T (lhsT layout)
        out = np.zeros((128, 128), np.float32)
        out[:C, :C] = M.T
        out[C:, C:] = M.T
        return out

    w1m = np.stack([bd(W1pr), bd(-W1pi), bd(W1pi), bd(W1pr)])
    w2m = np.stack([bd(W2r), bd(-W2i), bd(W2i), bd(W2r)])

    fdft, minv = _dft_matrices()
    bf = ml_dtypes.bfloat16

    sgnk = ((-1.0) ** np.arange(KF)).astype(np.float32)
    sgnrow = np.zeros(KP, np.float32)
    sgnrow[0:KF] = sgnk
    sgnrow[KF:K2] = sgnk

    dwt = np.asarray(inputs["dw_weight"], np.float32)
    dw_bc = np.tile(dwt, (2, 1)).astype(np.float32)          # [128, 11]
    dwdiag = np.zeros((11, 128, 128), np.float32)
    for u in range(11):
        np.fill_diagonal(dwdiag[u], dw_bc[:, u])
    brelu = np.full((128, 1), float(np.asarray(inputs["b_relu"])), np.float32)

    common = dict(
        fdft=fdft.astype(bf),
        minv=minv.astype(bf),
        w1m=w1m.astype(bf),
        w2m=w2m.astype(bf),
        brelu=brelu,
        dwdiag=dwdiag.astype(bf),
    )
    if not fast:
        S1 = (W1pr + 1j * W1pi).sum(1)
        bias = np.asarray(inputs["sr_bias_r"], np.float32) + 1j * np.asarray(
            inputs["sr_bias_i"], np.float32)
        B1 = (W1r + 1j * W1i) @ bias
        s1 = np.stack([
            np.tile(-S1.real, 2), np.tile(S1.imag, 2),
            np.tile(-S1.imag, 2), np.tile(-S1.real, 2)], 1).astype(np.float32)
        b1 = np.stack([np.tile(B1.real, 2), np.tile(B1.imag, 2)], 1).astype(np.float32)
        common.update(s1=s1, b1=b1)

    in_maps = []
    for r in range(NCORES):
        h0 = HB * r - HALO
        gidx = np.arange(h0, h0 + NH)
        valid = (gidx >= 0) & (gidx < H)
        gv = np.clip(gidx, 0, H - 1)

        xs = np.zeros((NH, BC, WP), np.float32)
        xs[valid, :, :W] = x[:, gv[valid], :].transpose(1, 0, 2)
        # transpose w <-> bc per chunk: [NH, 6, 128(w), 128(bc)]
        xT = np.ascontiguousarray(
            xs.reshape(NH, BC, NCHUNK, 128).transpose(0, 2, 3, 1)).astype(bf)

        def rowsel(d2):   # [H, KF] -> [1, NH*KF] with invalid rows zeroed
            out = np.zeros((NH, KF), np.float32)
            out[valid] = d2[gv[valid]]
            return out

        def glusel(d3):   # [C, H, KF] -> [NH, 128, KF]
            out = np.zeros((NH, BC, KF), np.float32)
            sel = d3[:, gv[valid], :].transpose(1, 0, 2)   # [nvalid, C, KF]
            out[valid] = np.concatenate([sel, sel], 1)
            return out

        m = dict(common)
        m.update(
            xT=xT,
            istd=rowsel(istd_d),
            glum=glusel(gm_d),
            glup=glusel(gp_d),
            sgnpre=np.broadcast_to(
                sgnrow * (1.0 if r == 0 else 0.0), (128, KP)).astype(bf).copy(),
            sgnpost=np.broadcast_to(
                sgnrow * (1.0 if r == NCORES - 1 else 0.0), (128, KP)).astype(bf).copy(),
        )
        if not fast:
            m.update(mtr=rowsel(mtr_d), mti=rowsel(mti_d), stdm=rowsel(stdm_d))
        in_maps.append(m)
    return in_maps


def kernel(**inputs):
    fast = bool(
        np.all(np.asarray(inputs["sr_mean_r"]) == 0)
        and np.all(np.asarray(inputs["sr_mean_i"]) == 0)
        and np.all(np.asarray(inputs["sr_bias_r"]) == 0)
        and np.all(np.asarray(inputs["sr_bias_i"]) == 0)
        and np.all(np.asarray(inputs["sr_std"]) == 1)
        and float(np.asarray(inputs["b_relu"])) == 0.0
    )
    if not fast:
        return _numpy_fallback(inputs)
    if ("prog", fast) not in _CACHE:
        _CACHE[("prog", fast)] = build_program(fast)
    nc = _CACHE[("prog", fast)]

    in_maps = _host_prep(inputs, fast)
    res = bass_utils.run_bass_kernel_spmd(
        nc, in_maps, core_ids=list(range(NCORES)),
        trace=bool(int(os.environ.get("KTRACE", "0"))),
    )
    kernel.last_results = res

    out = np.zeros((BC, H, W), np.float32)
    for r in range(NCORES):
        y = res.results[r]["y"]            # [HB, 128, W]
        out[:, HB * r:HB * (r + 1), :] = y.transpose(1, 0, 2)
    return out.reshape(B, C, H, W).astype(np.float32)


def _numpy_fallback(inputs):
    """Exact-math host fallback for the general (non-eval-buffer) case."""
    from numpy import fft as _fft
    x = np.asarray(inputs["x"], np.float32)
    rows = np.asarray(inputs["rows"]); cols = np.asarray(inputs["cols"])
    xf = _fft.rfft(x, axis=-1, norm="ortho")
    xm = xf[:, :, rows, cols]
    mean = (np.asarray(inputs["sr_mean_r"]) + 1j * np.asarray(inputs["sr_mean_i"]))[None, None]
    h = (xm - mean) / (1e-12 + np.asarray(inputs["sr_std"])[None, None])
    h = h * (np.asarray(inputs["sr_mags_r"]) + 1j * np.asarray(inputs["sr_mags_i"]))[None, :, None] \
        + (np.asarray(inputs["sr_bias_r"]) + 1j * np.asarray(inputs["sr_bias_i"]))[None, :, None]
    h = np.einsum("oi,bit->bot", np.asarray(inputs["w1_r"]) + 1j * np.asarray(inputs["w1_i"]), h)
    from scipy.special import erf as _erf
    r = np.abs(h) + float(np.asarray(inputs["b_relu"]))
    g = 0.5 * r * (1.0 + _erf(r / np.sqrt(2.0)))
    h = g * np.exp(1j * np.angle(h))
    h = np.einsum("oi,bit->bot", np.asarray(inputs["w2_r"]) + 1j * np.asarray(inputs["w2_i"]), h)
    gate = 1.0 / (1.0 + np.exp(-(np.abs(h) + np.asarray(inputs["glu_mags"])[None]))) \
        * np.exp(1j * (np.angle(h) + np.asarray(inputs["glu_phases"])[None]))
    xm = xm * gate
    xf2 = np.zeros_like(xf)
    xf2[:, :, rows, cols] = xm
    y = _fft.irfft(xf2, n=W, axis=-1, norm="ortho").astype(np.float32)
    gr = np.roll(np.flip(y, axis=2), W // 2, axis=3)
    ypad = np.concatenate([gr[:, :, -5:], y, gr[:, :, :5]], axis=2)
    dw = np.asarray(inputs["dw_weight"])
    z = np.zeros_like(y)
    for u in range(11):
        z += dw[None, :, u, None, None] * ypad[:, :, u:u + H, :]
    return (y + z).astype(np.float32)
